# revision 7
# baseline (speedup 1.0000x reference)
"""Trainium2 Bass kernel for nn_AttentionLayer_13383118095164.

Cross-attention layer: q = target @ Wq + bq; k/v = source/value @ Wk/Wv + bk/bv;
out = softmax(q k^T / 8) v @ Wo + bo.   B=4, L=2048, S=1024, D=1024, H=16, E=64.

Sharding (8 cores): core c = (batch b = c//2, head-group g = c%2 of 8 heads).
Megatron-style: Q/K/V column-split by head group, Wo row-split; the two
head-group partial outputs per batch are summed on the host.

Device layout is fully transposed (host pre-transposes inputs) so the kernel
needs zero on-device transposes:
  Q^T = Wq_g^T X_t^T   [512, L]     K^T = Wk_g^T X_s^T  [512, S]
  V   = X_v W v_g      [S, 512]     (V gets a ones column per head)
  S^T_h = K^T_h^T-contraction-> [S, L] tiles; exp on ACT; softmax denominator
  comes from the ones column of V during O^T_h = V_h^T exp(S^T_h) (M=65).
  out^T = Wo_g^T O     [1024, L]
Bias algebra: bk shifts all logits of a softmax row equally -> dropped; bv
contributes (bv @ Wo) to every output row -> folded into host constant; bq and
the 1/8 scale are fused into the Q copyback activation (host passes bq/8).

All matmuls run in float32r (full-rate fp32, ~1.5e-4 relative error).
"""

import numpy as np

import concourse.bass as bass
import concourse.tile as tile
from concourse import mybir
from concourse.bass_utils import run_bass_kernel_spmd

P = 128
D = 1024  # d_model
DL = 4096  # d_llm
L = 2048  # target length
S = 1024  # source length
MQ = 512  # per-core q/k/v dims (8 heads x 64)
E = 64
HG = 8  # heads per core
LC = 512  # l-chunk
N_LC = L // LC
SCALE = 0.125  # 1/sqrt(E)

F32R = mybir.dt.float32r
F32 = mybir.dt.float32


def _split_multi_waits(nc):
    """This walrus build rejects >1 sync wait per instruction: split extras
    onto single-wait NOPs on the same engine immediately before (same program
    order on the same queue => identical semantics)."""
    for f in nc.m.functions:
        for blk in f.blocks:
            new_insts = []
            for inst in blk.instructions:
                si = inst.sync_info
                if si is not None and si.on_wait and len(si.on_wait) > 1:
                    waits = list(si.on_wait)
                    for w in waits[:-1]:
                        nop = mybir.InstNoOp(
                            name=f"I-waitsplit-{nc.next_id()}", ins=[], outs=[]
                        )
                        nop.engine = inst.engine
                        nop.sync_info = mybir.SyncInfo(on_wait=[w], on_update=[])
                        new_insts.append(nop)
                    si.on_wait = [waits[-1]]
                new_insts.append(inst)
            blk.instructions[:] = new_insts


def build_nc(reps: int = 1):
    from contextlib import ExitStack

    nc = bass.Bass(trn_type="TRN2", target_bir_lowering=False, debug=False)

    xt = nc.dram_tensor("xt", [D, L], F32R, kind="ExternalInput")  # X_t^T
    xs = nc.dram_tensor("xs", [DL, S], F32R, kind="ExternalInput")  # X_s^T
    xv = nc.dram_tensor("xv", [DL, S], F32R, kind="ExternalInput")  # X_v^T
    wq = nc.dram_tensor("wq", [D, MQ], F32R, kind="ExternalInput")
    wk = nc.dram_tensor("wk", [DL, MQ], F32R, kind="ExternalInput")
    wv = nc.dram_tensor("wv", [DL, MQ], F32R, kind="ExternalInput")
    wo = nc.dram_tensor("wo", [MQ, D], F32R, kind="ExternalInput")
    bq = nc.dram_tensor("bq", [P, 4], F32, kind="ExternalInput")  # bq/8 as [p, mt]
    out = nc.dram_tensor("out", [D, L], F32, kind="ExternalOutput")  # out^T partial

    with tile.TileContext(nc) as tc, ExitStack() as ctx:
        const = ctx.enter_context(tc.tile_pool(name="const", bufs=1))
        resident = ctx.enter_context(tc.tile_pool(name="resident", bufs=1))
        stream = ctx.enter_context(tc.tile_pool(name="stream", bufs=3))
        stream2 = ctx.enter_context(tc.tile_pool(name="stream2", bufs=2))
        work = ctx.enter_context(tc.tile_pool(name="work", bufs=2))
        work1 = ctx.enter_context(tc.tile_pool(name="work1", bufs=1))
        psum = ctx.enter_context(tc.tile_pool(name="psum", bufs=1, space="PSUM"))

        ps_counter = [0]

        def ps_tile(name):
            t = psum.tile(
                [P, 512],
                F32,
                tag=f"ps{ps_counter[0] % 8}",
                name=f"{name}_{ps_counter[0]}",
            )
            ps_counter[0] += 1
            return t

        # ---- resident weights / constants ----
        wq_sb = resident.tile([P, D // P, MQ], F32R, name="wq_sb")
        nc.sync.dma_start(wq_sb[:], wq.ap().rearrange("(kt p) m -> p kt m", p=P))
        wo_sb = resident.tile([P, MQ // P, D], F32R, name="wo_sb")
        nc.sync.dma_start(wo_sb[:], wo.ap().rearrange("(kt p) d -> p kt d", p=P))
        bq_sb = const.tile([P, 4], F32, name="bq_sb")
        nc.sync.dma_start(bq_sb[:], bq.ap())
        ones64 = const.tile([1, E], F32R, name="ones64")
        nc.vector.memset(ones64[:].bitcast(F32), 1.0)

        kT = resident.tile([P, 4, S], F32R, name="kT")  # [p, mt, s]
        v_sb = resident.tile([P, 8, HG, E + 1], F32R, name="v_sb")  # [p, st, h, e|1]
        nc.vector.memset(v_sb[:, :, :, E : E + 1].bitcast(F32), 1.0)

        for _rep in range(reps):
            # ---- phase A: K^T = Wk_g^T @ X_s^T   [512, S] -> kT[p, mt, s] ----
            kacc = [ps_tile(f"kacc{i}") for i in range(8)]
            for kt in range(DL // P):
                wk_t = stream.tile([P, MQ], F32R, tag="wk_t", name="wk_t")
                nc.sync.dma_start(wk_t[:], wq_ap_slice(wk, kt))
                xs_t = stream2.tile([P, S], F32R, tag="xs_t", name="xs_t")
                nc.sync.dma_start(xs_t[:], xs.ap()[kt * P : (kt + 1) * P, :])
                for mt in range(4):
                    for sc in range(2):
                        nc.tensor.matmul(
                            kacc[mt * 2 + sc][:],
                            wk_t[:, mt * P : (mt + 1) * P],
                            xs_t[:, sc * 512 : (sc + 1) * 512],
                            start=(kt == 0),
                            stop=(kt == DL // P - 1),
                        )
            for mt in range(4):
                for sc in range(2):
                    nc.vector.tensor_copy(
                        kT[:, mt, sc * 512 : (sc + 1) * 512], kacc[mt * 2 + sc][:]
                    )

            # ---- phase B: V = X_v @ Wv_g   [S, 512] -> v_sb[p, st, h, 0:64] ----
            vacc = [ps_tile(f"vacc{i}") for i in range(8)]
            for kt in range(DL // P):
                wv_t = stream.tile([P, MQ], F32R, tag="wv_t", name="wv_t")
                nc.sync.dma_start(wv_t[:], wq_ap_slice(wv, kt))
                xv_t = stream2.tile([P, S], F32R, tag="xv_t", name="xv_t")
                nc.sync.dma_start(xv_t[:], xv.ap()[kt * P : (kt + 1) * P, :])
                for st in range(8):
                    nc.tensor.matmul(
                        vacc[st][:],
                        xv_t[:, st * P : (st + 1) * P],
                        wv_t[:, :],
                        start=(kt == 0),
                        stop=(kt == DL // P - 1),
                    )
            for st in range(8):
                nc.vector.tensor_copy(
                    v_sb[:, st, :, 0:E],
                    vacc[st][:].rearrange("p (h e) -> p h e", e=E),
                )

            # ---- phase C: per l-chunk ----
            for lc in range(N_LC):
                lsl = slice(lc * LC, (lc + 1) * LC)

                # C1: Q^T chunk [512, LC] -> qT[p, mt, l], fused bias+scale
                qacc = [ps_tile(f"qacc{i}") for i in range(4)]
                for kt in range(D // P):
                    xt_t = stream.tile([P, LC], F32R, tag="xt_t", name="xt_t")
                    nc.sync.dma_start(xt_t[:], xt.ap()[kt * P : (kt + 1) * P, lsl])
                    for mt in range(4):
                        nc.tensor.matmul(
                            qacc[mt][:],
                            wq_sb[:, kt, mt * P : (mt + 1) * P],
                            xt_t[:],
                            start=(kt == 0),
                            stop=(kt == D // P - 1),
                        )
                qT = work.tile([P, 4, LC], F32R, tag="qT", name="qT")
                for mt in range(4):
                    nc.scalar.activation(
                        qT[:, mt, :],
                        qacc[mt][:],
                        mybir.ActivationFunctionType.Identity,
                        bias=bq_sb[:, mt : mt + 1],
                        scale=SCALE,
                    )

                # C2: attention per head; oT[p, mt, l] = normalized O^T
                oT = work.tile([P, 4, LC], F32R, tag="oT", name="oT")
                for hp in range(4):  # head pairs (2*hp, 2*hp+1)
                    expS = [None, None]
                    for a in range(2):
                        expS[a] = work1.tile(
                            [P, 8, LC], F32R, tag=f"expS{a}", name=f"expS{a}"
                        )
                    for st in range(8):
                        for a in range(2):  # row-paired K=64 matmuls
                            pa = 64 * a
                            t = ps_tile(f"sc{st}_{a}")
                            nc.tensor.matmul(
                                t[:],
                                kT[pa : pa + 64, hp, st * P : (st + 1) * P],
                                qT[pa : pa + 64, hp, :],
                                start=True,
                                stop=True,
                            )
                            nc.scalar.activation(
                                expS[a][:, st, :],
                                t[:],
                                mybir.ActivationFunctionType.Exp,
                            )
                    for a in range(2):
                        h = 2 * hp + a
                        po = ps_tile(f"o{h}")
                        for st in range(8):
                            nc.tensor.matmul(
                                po[0 : E + 1, :],
                                v_sb[:, st, h, :],
                                expS[a][:, st, :],
                                start=(st == 0),
                                stop=(st == 7),
                            )
                        rcp = work.tile([1, LC], F32R, tag="rcp", name="rcp")
                        with nc.allow_low_precision(
                            reason="float32r keeps full fp32 storage; matmul rhs needs f32r dtype"
                        ):
                            nc.vector.reciprocal(rcp[:], po[E : E + 1, :])
                        pb = ps_tile(f"b{h}")
                        nc.tensor.matmul(
                            pb[0:E, :], ones64[:], rcp[:], start=True, stop=True
                        )
                        bsb = work.tile([E, LC], F32, tag="bsb", name="bsb")
                        nc.scalar.copy(bsb[:], pb[0:E, :])
                        nc.vector.tensor_mul(
                            oT[64 * a : 64 * a + 64, hp, :], po[0:E, :], bsb[:]
                        )

                # C3: out^T chunk [1024, LC] = Wo_g^T @ O
                for mt8 in range(8):
                    po_out = ps_tile(f"out{mt8}")
                    for kt4 in range(4):
                        nc.tensor.matmul(
                            po_out[:],
                            wo_sb[:, kt4, mt8 * P : (mt8 + 1) * P],
                            oT[:, kt4, :],
                            start=(kt4 == 0),
                            stop=(kt4 == 3),
                        )
                    stg = work.tile([P, LC], F32, tag="stg", name="stg")
                    nc.vector.tensor_copy(stg[:], po_out[:])
                    nc.sync.dma_start(out.ap()[mt8 * P : (mt8 + 1) * P, lsl], stg[:])

    _split_multi_waits(nc)
    return nc


def wq_ap_slice(w, kt):
    return w.ap()[kt * P : (kt + 1) * P, :]


_NC_CACHE = {}


def _get_nc(reps=1):
    if reps not in _NC_CACHE:
        _NC_CACHE[reps] = build_nc(reps)
    return _NC_CACHE[reps]


def make_in_maps(inputs):
    te = np.asarray(inputs["target_embedding"], np.float32)
    se = np.asarray(inputs["source_embedding"], np.float32)
    ve = np.asarray(inputs["value_embedding"], np.float32)
    Wq = np.asarray(inputs["Wq"], np.float32)
    Wk = np.asarray(inputs["Wk"], np.float32)
    Wv = np.asarray(inputs["Wv"], np.float32)
    Wo = np.asarray(inputs["Wo"], np.float32)
    bqv = np.asarray(inputs["bq"], np.float32)
    in_maps = []
    for core in range(8):
        b, g = divmod(core, 2)
        sl = slice(MQ * g, MQ * (g + 1))
        in_maps.append(
            {
                "xt": np.ascontiguousarray(te[b].T),
                "xs": np.ascontiguousarray(se[b].T),
                "xv": np.ascontiguousarray(ve[b].T),
                "wq": np.ascontiguousarray(Wq[:, sl]),
                "wk": np.ascontiguousarray(Wk[:, sl]),
                "wv": np.ascontiguousarray(Wv[:, sl]),
                "wo": np.ascontiguousarray(Wo[sl, :]),
                "bq": np.ascontiguousarray((bqv[sl] * SCALE).reshape(4, P).T),
            }
        )
    return in_maps


def assemble_output(results, inputs):
    bv = np.asarray(inputs["bv"], np.float32)
    bo = np.asarray(inputs["bo"], np.float32)
    Wo = np.asarray(inputs["Wo"], np.float32)
    corr = (bv @ Wo + bo).astype(np.float32)  # [D]
    out = np.empty((4, L, D), np.float32)
    for b in range(4):
        acc = results[2 * b]["out"] + results[2 * b + 1]["out"]  # [D, L]
        out[b] = acc.T + corr
    return out


def kernel(**inputs) -> np.ndarray:
    nc = _get_nc(1)
    in_maps = make_in_maps(inputs)
    res = run_bass_kernel_spmd(nc, in_maps, core_ids=list(range(8)))
    return assemble_output(res.results, inputs)


# revision 16
# speedup vs baseline: 657.5699x; 657.5699x over previous
"""Trainium2 Bass kernel for nn_AttentionLayer_13383118095164.

Cross-attention layer: q = target @ Wq + bq; k/v = source/value @ Wk/Wv + bk/bv;
out = softmax(q k^T / 8) v @ Wo + bo.   B=4, L=2048, S=1024, D=1024, H=16, E=64.

Sharding (8 cores): core c = (batch b = c//2, head-group g = c%2 of 8 heads).
Megatron-style: Q/K/V column-split by head group, Wo row-split; the two
head-group partial outputs per batch are summed on the host.

Device layout is fully transposed (host pre-transposes inputs) so the kernel
needs zero on-device transposes:
  Q^T = Wq_g^T X_t^T   [512, L]     K^T = Wk_g^T X_s^T  [512, S]
  V   = X_v Wv_g       [S, 512]     (V gets a ones column per head)
  S^T_h tiles [S, L]; exp on ACT; softmax denominator comes from the ones
  column of V during O^T_h = V_h^T exp(S^T_h) (M=65 matmuls).
  out^T = Wo_g^T O     [1024, L]
Bias algebra: bk shifts all logits of a softmax row equally -> dropped; bv
contributes (bv @ Wo) to every output row -> folded into host constant; bq and
the 1/8 scale are fused into the Q copyback activation (host passes bq/8).

All matmuls run in float32r (full-rate fp32 streaming, ~1.5e-4 rel error).
Phase order: Q-projection first (PE work that hides the K/V input DMA ramp),
then K^T, V, then per l-chunk a software-pipelined scores->exp->O chain with
one half-pass of lag so the PE never waits on the current exp batch.
"""

import numpy as np

import concourse.bass as bass
import concourse.tile as tile
from concourse import mybir
from concourse.bass_utils import run_bass_kernel_spmd

P = 128
D = 1024  # d_model
DL = 4096  # d_llm
L = 2048  # target length
S = 1024  # source length
MQ = 512  # per-core q/k/v dims (8 heads x 64)
E = 64
HG = 8  # heads per core
LC = 512  # l-chunk
N_LC = L // LC
SCALE = 0.125  # 1/sqrt(E)

F32R = mybir.dt.float32r
F32 = mybir.dt.float32


def _split_multi_waits(nc):
    """This walrus build rejects >1 sync wait per instruction: split extras
    onto single-wait NOPs on the same engine immediately before (same program
    order on the same queue => identical semantics)."""
    for f in nc.m.functions:
        for blk in f.blocks:
            new_insts = []
            for inst in blk.instructions:
                si = inst.sync_info
                if si is not None and si.on_wait and len(si.on_wait) > 1:
                    waits = list(si.on_wait)
                    for w in waits[:-1]:
                        nop = mybir.InstNoOp(
                            name=f"I-waitsplit-{nc.next_id()}", ins=[], outs=[]
                        )
                        nop.engine = inst.engine
                        nop.sync_info = mybir.SyncInfo(on_wait=[w], on_update=[])
                        new_insts.append(nop)
                    si.on_wait = [waits[-1]]
                new_insts.append(inst)
            blk.instructions[:] = new_insts


def build_nc(reps: int = 1, phases: str = "ALL"):
    from contextlib import ExitStack

    nc = bass.Bass(trn_type="TRN2", target_bir_lowering=False, debug=False)

    xt = nc.dram_tensor("xt", [D, L], F32R, kind="ExternalInput")  # X_t^T
    xs = nc.dram_tensor("xs", [DL, S], F32R, kind="ExternalInput")  # X_s^T
    xv = nc.dram_tensor("xv", [DL, S], F32R, kind="ExternalInput")  # X_v^T
    wq = nc.dram_tensor("wq", [D, MQ], F32R, kind="ExternalInput")
    wk = nc.dram_tensor("wk", [DL, MQ], F32R, kind="ExternalInput")
    wv = nc.dram_tensor("wv", [DL, MQ], F32R, kind="ExternalInput")
    wo = nc.dram_tensor("wo", [MQ, D], F32R, kind="ExternalInput")
    bq = nc.dram_tensor("bq", [P, 4], F32, kind="ExternalInput")  # bq/8 as [p, mt]
    out = nc.dram_tensor("out", [D, L], F32, kind="ExternalOutput")  # out^T partial

    with tile.TileContext(nc) as tc, ExitStack() as ctx:
        const = ctx.enter_context(tc.tile_pool(name="const", bufs=1))
        resident = ctx.enter_context(tc.tile_pool(name="resident", bufs=1))
        stream = ctx.enter_context(tc.tile_pool(name="stream", bufs=3))
        stream2 = ctx.enter_context(tc.tile_pool(name="stream2", bufs=2))
        psum = ctx.enter_context(tc.tile_pool(name="psum", bufs=1, space="PSUM"))

        ps_counter = [0]

        def ps_big(name, tag):
            t = psum.tile(
                [P, 1024], F32, tag=f"big{tag % 4}", name=f"{name}_{ps_counter[0]}"
            )
            ps_counter[0] += 1
            return t

        def ps_halves(name, n, tag0=0):
            bigs = [ps_big(f"{name}{i}", tag0 + i) for i in range((n + 1) // 2)]
            return [
                bigs[i // 2][:, 512 * (i % 2) : 512 * (i % 2 + 1)] for i in range(n)
            ]

        # ---- resident weights / constants ----
        wo_sb = resident.tile([P, MQ // P, D], F32R, name="wo_sb")
        nc.sync.dma_start(wo_sb[:], wo.ap().rearrange("(kt p) d -> p kt d", p=P))
        bq_sb = const.tile([P, 4], F32, name="bq_sb")
        nc.sync.dma_start(bq_sb[:], bq.ap())
        ones64 = const.tile([1, E], F32R, name="ones64")
        nc.vector.memset(ones64[:].bitcast(F32), 1.0)

        kT = resident.tile([P, 4, S], F32R, name="kT")  # [p, mt, s]
        v_sb = resident.tile([P, 8, HG, E + 1], F32R, name="v_sb")  # [p, st, h, e|1]
        nc.vector.memset(v_sb[:, :, :, E : E + 1].bitcast(F32), 1.0)
        qT = resident.tile([P, 4, N_LC, LC], F32R, name="qT")  # [p, mt, lc, l]

        if phases == "C":
            nc.vector.memset(kT[:].bitcast(F32), 0.01)
            nc.vector.memset(v_sb[:].bitcast(F32), 0.01)
            nc.vector.memset(qT[:].bitcast(F32), 0.01)

        def _body(wq_pool, work):
            if phases in ("ALL", "AB"):
                _build_q(nc, ps_halves, stream, wq_pool, qT, wq, xt, bq_sb)
                _build_kv(nc, ps_halves, stream, stream2, kT, v_sb, wk, xs, wv, xv)
            if phases in ("ALL", "C"):
                _build_attn(
                    nc, ps_big, ps_halves, work, kT, v_sb, qT, wo_sb, ones64, out
                )

        if reps == 1:
            with (
                tc.tile_pool(name="wq_pool", bufs=1) as wq_pool,
                tc.tile_pool(name="work", bufs=2) as work,
            ):
                _body(wq_pool, work)
        else:
            with tc.For_i(0, reps, 1):
                with (
                    tc.tile_pool(name="wq_pool", bufs=1) as wq_pool,
                    tc.tile_pool(name="work", bufs=2) as work,
                ):
                    _body(wq_pool, work)

    _split_multi_waits(nc)
    return nc


def _build_q(nc, ps_halves, stream, wq_pool, qT, wq, xt, bq_sb):
    """Q^T = Wq_g^T @ X_t^T -> qT[p, mt, lc, l], fused bias + 1/8 scale.

    Runs first: pure PE work whose inputs (wq, xt: 10MB) are small, hiding the
    DMA ramp of the 48MB K/V input stream behind it."""
    wq_sb = wq_pool.tile([P, D // P, MQ], F32R, name="wq_sb")
    nc.sync.dma_start(wq_sb[:], wq.ap().rearrange("(kt p) m -> p kt m", p=P))
    for lc in range(N_LC):
        qacc = ps_halves("qacc", 4, tag0=2 * (lc % 2))
        for kt in range(D // P):
            xt_t = stream.tile([P, LC], F32R, tag="xt_t", name="xt_t")
            nc.sync.dma_start(
                xt_t[:], xt.ap()[kt * P : (kt + 1) * P, lc * LC : (lc + 1) * LC]
            )
            for mt in range(4):
                nc.tensor.matmul(
                    qacc[mt][:],
                    wq_sb[:, kt, mt * P : (mt + 1) * P],
                    xt_t[:],
                    start=(kt == 0),
                    stop=(kt == D // P - 1),
                )
        for mt in range(4):
            nc.scalar.activation(
                qT[:, mt, lc, :],
                qacc[mt][:],
                mybir.ActivationFunctionType.Identity,
                bias=bq_sb[:, mt : mt + 1],
                scale=SCALE,
            )


def _build_kv(nc, ps_halves, stream, stream2, kT, v_sb, wk, xs, wv, xv):
    # ---- K^T = Wk_g^T @ X_s^T   [512, S] -> kT[p, mt, s] ----
    kacc = ps_halves("kacc", 8)
    for kt in range(DL // P):
        wk_t = stream.tile([P, MQ], F32R, tag="wk_t", name="wk_t")
        nc.sync.dma_start(wk_t[:], wk.ap()[kt * P : (kt + 1) * P, :])
        xs_t = stream2.tile([P, S], F32R, tag="xs_t", name="xs_t")
        nc.sync.dma_start(xs_t[:], xs.ap()[kt * P : (kt + 1) * P, :])
        for mt in range(4):
            for sc in range(2):
                nc.tensor.matmul(
                    kacc[mt * 2 + sc][:],
                    wk_t[:, mt * P : (mt + 1) * P],
                    xs_t[:, sc * 512 : (sc + 1) * 512],
                    start=(kt == 0),
                    stop=(kt == DL // P - 1),
                )
    for mt in range(4):
        for sc in range(2):
            nc.vector.tensor_copy(
                kT[:, mt, sc * 512 : (sc + 1) * 512], kacc[mt * 2 + sc][:]
            )

    # ---- V = X_v @ Wv_g   [S, 512] -> v_sb[p, st, h, 0:64] ----
    vacc = ps_halves("vacc", 8)
    for kt in range(DL // P):
        wv_t = stream.tile([P, MQ], F32R, tag="wv_t", name="wv_t")
        nc.sync.dma_start(wv_t[:], wv.ap()[kt * P : (kt + 1) * P, :])
        xv_t = stream2.tile([P, S], F32R, tag="xv_t", name="xv_t")
        nc.sync.dma_start(xv_t[:], xv.ap()[kt * P : (kt + 1) * P, :])
        for st in range(8):
            nc.tensor.matmul(
                vacc[st][:],
                xv_t[:, st * P : (st + 1) * P],
                wv_t[:, :],
                start=(kt == 0),
                stop=(kt == DL // P - 1),
            )
    for st in range(8):
        nc.vector.tensor_copy(
            v_sb[:, st, :, 0:E], vacc[st][:].rearrange("p (h e) -> p h e", e=E)
        )


def _build_attn(nc, ps_big, ps_halves, work, kT, v_sb, qT, wo_sb, ones64, out):
    """Software-pipelined scores -> exp -> O^T -> normalize -> out-projection.

    Flat stage stream over (lc, head-pair, half): each stage issues the score
    matmuls + exp for its half, then the O^T accumulation for the PREVIOUS
    stage (whose exps had a full stage of ACT time to finish). Normalization
    runs when a pair's second half completes; the out-projection for an
    l-chunk is issued when its last pair finishes."""
    E1 = E + 1

    stages = [(lc, hp, g) for lc in range(N_LC) for hp in range(4) for g in range(2)]
    ot_bigs = {}  # (lc, hp) -> [tile_a0, tile_a1]
    oTs = {}  # lc -> oT tile

    def issue_scores(lc, hp, g):
        expS = []
        for a in range(2):
            expS.append(
                work.tile([P, 2, 2, LC], F32R, tag=f"expS{a}", name=f"expS{a}")
            )
        for stp in range(2):
            tt = [ps_big(f"sc{stp}_{a}", a) for a in range(2)]
            for half in range(2):
                st = 4 * g + 2 * stp + half
                for a in range(2):  # row-paired K=64 matmuls
                    pa = 64 * a
                    nc.tensor.matmul(
                        tt[a][:, 512 * half : 512 * (half + 1)],
                        kT[pa : pa + 64, hp, st * P : (st + 1) * P],
                        qT[pa : pa + 64, hp, lc, :],
                        start=True,
                        stop=True,
                    )
            for a in range(2):
                nc.scalar.activation(
                    expS[a][:, stp, :, :],
                    tt[a][:].rearrange("p (t l) -> p t l", l=LC),
                    mybir.ActivationFunctionType.Exp,
                )
        return expS

    def issue_o(lc, hp, g, expS):
        if g == 0:
            ot_bigs[(lc, hp)] = [ps_big(f"o{hp}_{a}", 2 + a) for a in range(2)]
        for a in range(2):
            h = 2 * hp + a
            po = ot_bigs[(lc, hp)][a][:, 0:512]
            for stq in range(4):
                st = 4 * g + stq
                nc.tensor.matmul(
                    po[0:E1, :],
                    v_sb[:, st, h, :],
                    expS[a][:, stq // 2, stq % 2, :],
                    start=(st == 0),
                    stop=(st == 7),
                    skip_group_check=True,
                )
        if g == 1:
            if lc not in oTs:
                oTs[lc] = work.tile([P, 4, LC], F32R, tag="oT", name="oT")
            oT = oTs[lc]
            for a in range(2):
                h = 2 * hp + a
                tb = ot_bigs[(lc, hp)][a]
                po, pb = tb[:, 0:512], tb[:, 512:1024]
                rcp = work.tile([1, LC], F32R, tag="rcp", name="rcp")
                with nc.allow_low_precision(
                    reason="float32r keeps full fp32 storage; matmul rhs dtype"
                ):
                    nc.vector.reciprocal(rcp[:], po[E : E + 1, :])
                nc.tensor.matmul(pb[0:E, :], ones64[:], rcp[:], start=True, stop=True)
                bsb = work.tile([E, LC], F32, tag="bsb", name="bsb")
                nc.vector.tensor_copy(bsb[:], pb[0:E, :])
                nc.vector.tensor_mul(
                    oT[64 * a : 64 * a + 64, hp, :], po[0:E, :], bsb[:]
                )
            del ot_bigs[(lc, hp)]
            if hp == 3:
                issue_outproj(lc, oT)
                del oTs[lc]

    def issue_outproj(lc, oT):
        out_halves = ps_halves("outp", 8)
        for mt8 in range(8):
            po_out = out_halves[mt8]
            for kt4 in range(4):
                nc.tensor.matmul(
                    po_out[:],
                    wo_sb[:, kt4, mt8 * P : (mt8 + 1) * P],
                    oT[:, kt4, :],
                    start=(kt4 == 0),
                    stop=(kt4 == 3),
                )
            stg = work.tile([P, LC], F32, tag="stg", name="stg")
            nc.vector.tensor_copy(stg[:], po_out[:])
            nc.sync.dma_start(
                out.ap()[mt8 * P : (mt8 + 1) * P, lc * LC : (lc + 1) * LC], stg[:]
            )

    pending = None
    for lc, hp, g in stages:
        expS = issue_scores(lc, hp, g)
        if pending is not None:
            issue_o(*pending)
        pending = (lc, hp, g, expS)
    issue_o(*pending)


_NC_CACHE = {}


def _get_nc(reps=1):
    if reps not in _NC_CACHE:
        _NC_CACHE[reps] = build_nc(reps)
    return _NC_CACHE[reps]


def make_in_maps(inputs):
    te = np.asarray(inputs["target_embedding"], np.float32)
    se = np.asarray(inputs["source_embedding"], np.float32)
    ve = np.asarray(inputs["value_embedding"], np.float32)
    Wq = np.asarray(inputs["Wq"], np.float32)
    Wk = np.asarray(inputs["Wk"], np.float32)
    Wv = np.asarray(inputs["Wv"], np.float32)
    Wo = np.asarray(inputs["Wo"], np.float32)
    bqv = np.asarray(inputs["bq"], np.float32)
    in_maps = []
    for core in range(8):
        b, g = divmod(core, 2)
        sl = slice(MQ * g, MQ * (g + 1))
        in_maps.append(
            {
                "xt": np.ascontiguousarray(te[b].T),
                "xs": np.ascontiguousarray(se[b].T),
                "xv": np.ascontiguousarray(ve[b].T),
                "wq": np.ascontiguousarray(Wq[:, sl]),
                "wk": np.ascontiguousarray(Wk[:, sl]),
                "wv": np.ascontiguousarray(Wv[:, sl]),
                "wo": np.ascontiguousarray(Wo[sl, :]),
                "bq": np.ascontiguousarray((bqv[sl] * SCALE).reshape(4, P).T),
            }
        )
    return in_maps


def assemble_output(results, inputs):
    bv = np.asarray(inputs["bv"], np.float32)
    bo = np.asarray(inputs["bo"], np.float32)
    Wo = np.asarray(inputs["Wo"], np.float32)
    corr = (bv @ Wo + bo).astype(np.float32)  # [D]
    out = np.empty((4, L, D), np.float32)
    for b in range(4):
        acc = results[2 * b]["out"] + results[2 * b + 1]["out"]  # [D, L]
        out[b] = acc.T + corr
    return out


def kernel(**inputs) -> np.ndarray:
    nc = _get_nc(1)
    in_maps = make_in_maps(inputs)
    res = run_bass_kernel_spmd(nc, in_maps, core_ids=list(range(8)))
    return assemble_output(res.results, inputs)
